# revision 5
# baseline (speedup 1.0000x reference)
"""Trainium2 Bass kernel for the B-spline (KAN-style) layer.

Math: out[b,o] = sum_{i,k} B3_k(t(b,i)) * coeff[i,o,k], where t = tanh(x)
mapped to knot coordinates t = (tanh(x) - grid[0]) / h in (3, 8), and B3 are
cubic B-spline bases over uniform integer knots.

Key transformation: each cubic B-spline basis is an exact linear combination
of the truncated cubes L_m = relu(t-m)^3 and the raw cubes c_m = (t-m)^3,
m in {4,5,6,7} (divided-difference identity; R_m = (m-t)_+^3 = L_m - c_m is
eliminated so only TWO planes per knot are computed on device):
    a_m = 2.5*tanh(x) + (5.5 - m)     (DVE tensor_scalar)
    s_m = Square(a_m)                 (ACT engine, affine fused into the op)
    c_m = s_m * a_m                   (DVE tensor_mul)
    L_m = max(c_m, 0)                 (DVE tensor_scalar_max)
The 8x8 basis-change matrix (planes [L4,c4,L5,c5,L6,c6,L7,c7]) is folded into
the coefficient tensor on the host.

Then a dense fp16 matmul: out[o,b] = sum_{(i,plane)} C3[(i,plane),o] * rho[(i,plane),b]
with contraction K = 512*8 = 4096, run on the PE at full fp16 rate.

Schedule notes (from trace analysis):
  - the 256-matmul stream runs at the theoretical 216.7ns/MM once started, so
    the wins are starting it earlier and shortening the tail;
  - plane production is split ACT (tanh+squares) / DVE (affine+cube+relu) so
    neither engine gates the stream;
  - chunk 0 half 0 is produced at half-batch width (FD=512) to minimize
    first-matmul latency; everything else at full width (FD=1024), which also
    means pass 2 (second batch half) needs no further elementwise work;
  - xt and out are stored tile-contiguous in DRAM so DMAs aren't
    descriptor/stride-bound.

Sharding: data-parallel over batch (8192 -> 8 x 1024); coefficients replicated.
"""

from contextlib import ExitStack, nullcontext

import numpy as np

import concourse.bass as bass
import concourse.mybir as mybir
import concourse.tile as tile
from concourse.bass_utils import run_bass_kernel_spmd
from concourse.tile import add_dep_helper
from concourse.vector_clock import ScopedClock

F32 = mybir.dt.float32
F16 = mybir.dt.float16

N_CORES = 8
B_FULL = 8192
B_SHARD = B_FULL // N_CORES  # 1024
I_FEAT = 512
O_FEAT = 512
NPLANES = 8
NCHUNK = I_FEAT // 128  # 4
ALU = mybir.AluOpType
AF = mybir.ActivationFunctionType

N_WARMUP = 10

# ---------------------------------------------------------------------------
# Workaround for walrus "Too many sync wait commands" on the TileContext final
# Drain: spread the accumulated semaphore waits across single-wait nofuse NOPs
# on the sync engine, then emit a bare drain + the usual barrier/cleanup.
_MAXW = 1


def _patched_drain_and_barrier(self, tick_clock, wait_clock):
    nc = self.nc
    probe = nc.sync.nop(nofuse=True)
    wait_clock.add_sem_waits(probe.ins, ScopedClock({None: tick_clock.global_clock}))
    si = probe.ins.sync_info
    waits = list(si.on_wait) if si and si.on_wait else []
    if len(waits) > _MAXW:
        si.on_wait = waits[:_MAXW]
        rest = waits[_MAXW:]
        while rest:
            chunk, rest = rest[:_MAXW], rest[_MAXW:]
            n2 = nc.sync.nop(nofuse=True)
            s2 = n2.ins.sync_info
            if s2 is None:
                n2.ins.sync_info = mybir.SyncInfo(on_wait=chunk, on_update=[])
            else:
                s2.on_wait = chunk
    nc.sync.drain()
    nc.all_engine_barrier()
    assert self.sems is not None
    popped = nc._tile_sem_poison_stack.pop()
    assert popped is self._sem_poison
    nc.clear_and_free_semaphores(list(self.sems.allocated().values()))


tile.TileContext._drain_and_barrier = _patched_drain_and_barrier


def _split_all_waits(nc: bass.Bass) -> None:
    """This image's walrus rejects instructions carrying more than one sync
    wait. Hoist all but the last wait of each instruction onto fresh NoOps on
    the same engine immediately before it (in-order issue makes this
    equivalent, merely slightly stronger synchronization)."""
    cnt = 0
    for f in nc.m.functions:
        for bb in f.blocks:
            out = []
            changed = False
            for inst in bb.instructions:
                si = inst.sync_info
                waits = list(si.on_wait) if si and si.on_wait else []
                if len(waits) > 1:
                    changed = True
                    for w in waits[:-1]:
                        nop = mybir.InstNoOp(name=f"waitsplit-{cnt}", ins=[], outs=[])
                        cnt += 1
                        nop.engine = inst.engine
                        nop.sync_info = mybir.SyncInfo(on_wait=[w], on_update=[])
                        out.append(nop)
                    si.on_wait = [waits[-1]]
                out.append(inst)
            if changed:
                bb.instructions = out


# ---------------------------------------------------------------------------


def _build_nc(t_scale: float, t_bias: float) -> bass.Bass:
    """Build the per-core Bass program.

    Per-core I/O (all tile-contiguous in DRAM):
      xt : (4, 2, 128, 512) f16   x^T shard (chunk, batch-half, part, col)
      c3 : (4, 128, 4096) f16     folded coefficients [chunk, part, plane*512+o]
      out: (4, 2, 128, 512) f32   output (o-chunk, batch-half, part, col)
    """
    nc = bass.Bass()
    # Register the Square-activation bias constants (t_bias - m) as const APs
    # (the activation instruction requires per-partition bias tensors).
    for m in (4, 5, 6, 7):
        v = float(t_bias - m)
        tens = nc.alloc_sbuf_tensor(f"const-bias-{m}", [128, 1], F32)
        nc.gpsimd.memset(tens.ap(), v)
        nc.const_aps.aps[(F32, v)] = tens.ap()
    nc.all_engine_barrier()
    xt = nc.declare_dram_parameter("xt", [NCHUNK, 2, 128, B_SHARD // 2], F16,
                                   isOutput=False)
    c3 = nc.declare_dram_parameter(
        "c3", [NCHUNK, 128, NPLANES * O_FEAT], F16, isOutput=False
    )
    out = nc.declare_dram_parameter("out", [NCHUNK, 2, 128, B_SHARD // 2], F32,
                                    isOutput=True)

    with tile.TileContext(nc) as tc, ExitStack() as ctx:
        c3_pool = ctx.enter_context(tc.tile_pool(name="c3", bufs=1))
        xin_pool = ctx.enter_context(tc.tile_pool(name="xin", bufs=1))
        xn_pool = ctx.enter_context(tc.tile_pool(name="xn", bufs=1))
        sq_pool = ctx.enter_context(tc.tile_pool(name="sq", bufs=3))
        aff_pool = ctx.enter_context(tc.tile_pool(name="aff", bufs=3))
        rho_pool = ctx.enter_context(tc.tile_pool(name="rho", bufs=1))
        ps_pool = ctx.enter_context(
            tc.tile_pool(name="ps", bufs=1, space=bass.MemorySpace.PSUM)
        )
        ost_pool = ctx.enter_context(tc.tile_pool(name="ost", bufs=1))

        BHALF = B_SHARD // 2  # 512

        # Dummy 1-column activation with no deps: hoists the ~1.3us ACT
        # table load to kernel start, off the tanh critical path.
        dummy = xn_pool.tile([128, 1], F16, tag="dummy")
        nc.gpsimd.memset(dummy[:], 0.0)
        nc.scalar.activation(dummy[:], dummy[:], AF.Tanh)

        # c3 streams on the gpsimd HWDGE ring (idle engine) so triggers don't
        # delay the ACT queue (table load + tanh) or the sync queue (xt).
        # Chunk 0 is split so the first K-slice (512 cols) lands early.
        c3_sb = []
        for c in range(NCHUNK):
            ct = c3_pool.tile([128, NPLANES * O_FEAT], F16, tag=f"c3_{c}")
            if c == 0:
                nc.gpsimd.dma_start(ct[:, :O_FEAT], c3[0][:, :O_FEAT])
                nc.gpsimd.dma_start(ct[:, O_FEAT:], c3[0][:, O_FEAT:])
            else:
                nc.gpsimd.dma_start(ct[:], c3[c])
            c3_sb.append(ct)

        # PE warm-up: zero matmuls with no data deps at kernel start, so the
        # HAM clock-gate reaches 8/8 before the first real matmul.
        wz = c3_pool.tile([128, 512], F16, tag="warmz")
        nc.gpsimd.memset(wz[:], 0.0)

        # rho[h][c][r] planes interleaved [L4, c4, L5, c5, ...] (basis change
        # folded into c3 on the host). Chunk 0 is produced at half-batch
        # width; chunks 1-3 at full width (slices feed both matmul passes).
        rho = [[[None] * NPLANES for _ in range(NCHUNK)] for _ in range(2)]

        def produce_unit(xn_t, c, mi, m, fd_half, prio_ctx, prev_end):
            """Emit one m-unit (two planes) for chunk c. fd_half: 0/1 for a
            half-batch unit on that half, None for full-batch."""
            beta = t_bias - m
            if fd_half is None:
                shape = [128, 2 * BHALF]
            else:
                shape = [128, BHALF]
            with prio_ctx:
                sq = sq_pool.tile(shape, F16, tag="sqh" if fd_half is not None else "sqf")
                sq_i = nc.scalar.activation(
                    sq[:], xn_t[:], AF.Square, bias=float(beta), scale=t_scale
                )
                a = aff_pool.tile(shape, F16, tag="ah" if fd_half is not None else "af")
                a_i = nc.vector.tensor_scalar(
                    a[:], xn_t[:], t_scale, beta, ALU.mult, ALU.add
                )
                if prev_end is not None:
                    # Order-only edge: keep the scheduler from interleaving
                    # the next m's ops into the chain that gates the matmul
                    # stream.
                    add_dep_helper(a_i.ins, prev_end.ins, sync=False,
                                   reason="first-chain order")
                cc = rho_pool.tile(shape, F16, tag=f"rho_c{c}_{2 * mi + 1}_{fd_half}")
                nc.vector.tensor_mul(cc[:], sq[:], a[:])
                lp = rho_pool.tile(shape, F16, tag=f"rho_c{c}_{2 * mi}_{fd_half}")
                lp_i = nc.vector.tensor_scalar_max(lp[:], cc[:], 0.0)
            return lp, cc, lp_i

        # --- chunk 0, half 0: half-batch production, priority-pinned ---
        xt00 = xin_pool.tile([128, BHALF], F16, tag="xt00")
        nc.sync.dma_start(xt00[:], xt[0][0])
        xn00 = xn_pool.tile([128, BHALF], F16, tag="xn00")
        nc.scalar.activation(xn00[:], xt00[:], AF.Tanh)
        half_planes = [[None] * NPLANES for _ in range(2)]  # chunk0 per half
        prev = None
        for mi, m in enumerate((4, 5, 6, 7)):
            prio = tc.high_priority() if mi == 0 else nullcontext()
            lp, cc, prev = produce_unit(xn00, 0, mi, m, 0, prio,
                                        prev if mi == 1 else None)
            half_planes[0][2 * mi] = lp
            half_planes[0][2 * mi + 1] = cc

        # --- chunks 1-3: full-batch production ---
        for c in range(1, NCHUNK):
            xtc = xin_pool.tile([128, 2 * BHALF], F16, tag=f"xt{c}")
            nc.sync.dma_start(xtc[:, :BHALF], xt[c][0])
            nc.sync.dma_start(xtc[:, BHALF:], xt[c][1])
            xnc = xn_pool.tile([128, 2 * BHALF], F16, tag=f"xn{c}")
            nc.scalar.activation(xnc[:], xtc[:], AF.Tanh)
            for mi, m in enumerate((4, 5, 6, 7)):
                lp, cc, _ = produce_unit(xnc, c, mi, m, None, nullcontext(), None)
                for h in range(2):
                    rho[h][c][2 * mi] = lp[:, h * BHALF:(h + 1) * BHALF]
                    rho[h][c][2 * mi + 1] = cc[:, h * BHALF:(h + 1) * BHALF]

        # --- chunk 0, half 1: half-batch production (needed only by pass 2) ---
        xt01 = xin_pool.tile([128, BHALF], F16, tag="xt01")
        nc.sync.dma_start(xt01[:], xt[0][1])
        xn01 = xn_pool.tile([128, BHALF], F16, tag="xn01")
        nc.scalar.activation(xn01[:], xt01[:], AF.Tanh)
        for mi, m in enumerate((4, 5, 6, 7)):
            lp, cc, _ = produce_unit(xn01, 0, mi, m, 1, nullcontext(), None)
            half_planes[1][2 * mi] = lp
            half_planes[1][2 * mi + 1] = cc
        for h in range(2):
            for r in range(NPLANES):
                rho[h][0][r] = half_planes[h][r][:]

        # Dense matmul: 8 PSUM tiles (o_chunk x b_half) accumulated over all
        # 32 (chunk, plane) K-slices. Pass h=0 is K-major (dense PE stream
        # consuming planes in production order); its PSUM eviction + output
        # DMA overlap pass h=1. Pass h=1 is o-major so each o-tile's
        # eviction + DMA trickle out during the remaining matmuls.
        ps = [
            [
                ps_pool.tile([128, 512], F32, tag=f"ps{o}_{h}", name=f"ps{o}_{h}")
                for h in range(2)
            ]
            for o in range(NCHUNK)
        ]
        NK = NCHUNK * NPLANES

        for _ in range(N_WARMUP):
            nc.tensor.matmul(
                ps[0][0][:64, :], wz[:, :64], wz[:], start=True, stop=True
            )

        def emit_copy_out(o, h):
            # Copies alternate ACT/DVE; output DMA triggers alternate the
            # sync/gpsimd HWDGE rings so the tail's trigger issue
            # parallelizes instead of queueing.
            ot = ost_pool.tile([128, 512], F32, tag=f"ot{o}_{h}", name=f"ot{o}_{h}")
            if o % 2 == 0:
                nc.scalar.activation(ot[:], ps[o][h][:], AF.Copy)
            else:
                nc.vector.tensor_copy(ot[:], ps[o][h][:])
            eng = nc.sync if o % 2 == 0 else nc.gpsimd
            eng.dma_start(out[o][h], ot[:])

        for kk in range(NK):
            c, r = divmod(kk, NPLANES)
            rt = rho[0][c][r]
            for o in range(4):
                lhsT = c3_sb[c][:, r * O_FEAT + o * 128 : r * O_FEAT + (o + 1) * 128]
                nc.tensor.matmul(
                    ps[o][0][:], lhsT, rt, start=(kk == 0), stop=(kk == NK - 1)
                )
        for o in range(4):
            emit_copy_out(o, 0)
        KTAIL = NK - 4
        for kk in range(KTAIL):
            c, r = divmod(kk, NPLANES)
            rt = rho[1][c][r]
            for o in range(4):
                lhsT = c3_sb[c][:, r * O_FEAT + o * 128 : r * O_FEAT + (o + 1) * 128]
                nc.tensor.matmul(
                    ps[o][1][:], lhsT, rt, start=(kk == 0), stop=False
                )
        for o in range(4):
            for kk in range(KTAIL, NK):
                c, r = divmod(kk, NPLANES)
                lhsT = c3_sb[c][:, r * O_FEAT + o * 128 : r * O_FEAT + (o + 1) * 128]
                nc.tensor.matmul(
                    ps[o][1][:],
                    lhsT,
                    rho[1][c][r],
                    start=False,
                    stop=(kk == NK - 1),
                )
            emit_copy_out(o, 1)
    _split_all_waits(nc)
    return nc


# Basis-change: with L_m = (t-m)_+^3, R_m = (m-t)_+^3, c_m = (t-m)^3 = L_m - R_m,
# B3[j](t) = sum_m [WL[j,m]*L_m + WR[j,m]*R_m]/6
#          = sum_m [(WL+WR)[j,m]*L_m - WR[j,m]*c_m]/6
# (binomial divided-difference weights).
_WL = np.array(
    [
        [0, 0, 0, 0],
        [0, 0, 0, 0],
        [0, 0, 0, 0],
        [0, 0, 0, 0],
        [1, -4, 6, -4],
        [0, 1, -4, 6],
        [0, 0, 1, -4],
        [0, 0, 0, 1],
    ],
    dtype=np.float64,
)
_WR = np.array(
    [
        [1, 0, 0, 0],
        [-4, 1, 0, 0],
        [6, -4, 1, 0],
        [-4, 6, -4, 1],
        [0, 0, 0, 0],
        [0, 0, 0, 0],
        [0, 0, 0, 0],
        [0, 0, 0, 0],
    ],
    dtype=np.float64,
)

_nc_cache: dict = {}


def _prepare(x: np.ndarray, coefficients: np.ndarray, grid: np.ndarray):
    x = np.asarray(x, dtype=np.float32)
    coefficients = np.asarray(coefficients, dtype=np.float32)
    grid = np.asarray(grid, dtype=np.float32)

    # Knot-coordinate transform t = (tanh(x) - grid[0]) / h (uniform grid).
    h = float(grid[-1] - grid[0]) / (len(grid) - 1)
    t_scale = 1.0 / h
    t_bias = -float(grid[0]) / h  # t = t_scale * xn + t_bias; here 2.5, 5.5

    key = (round(t_scale, 9), round(t_bias, 9))
    if key not in _nc_cache:
        _nc_cache[key] = _build_nc(t_scale, t_bias)
    nc = _nc_cache[key]

    # Host-side coefficient fold: C3[i, r, o] = sum_j coeff[i,o,j] * W2[j,r],
    # planes interleaved [L4, c4, L5, c5, ...].
    w2 = np.zeros((8, 8), dtype=np.float64)
    for mi in range(4):
        w2[:, 2 * mi] = (_WL[:, mi] + _WR[:, mi]) / 6.0
        w2[:, 2 * mi + 1] = -_WR[:, mi] / 6.0
    c3f = np.einsum("ioj,jr->iro", coefficients.astype(np.float64), w2)
    c3_arr = np.ascontiguousarray(
        c3f.reshape(NCHUNK, 128, NPLANES, O_FEAT)
        .reshape(NCHUNK, 128, NPLANES * O_FEAT)
        .astype(np.float16)
    )

    # xt tile-contiguous: (chunk, half, part, col) per core.
    xt = x.T.astype(np.float16)  # (512, 8192)
    in_maps = []
    for core in range(N_CORES):
        xs = xt[:, core * B_SHARD : (core + 1) * B_SHARD]  # (512, 1024)
        xtc = np.ascontiguousarray(
            xs.reshape(NCHUNK, 128, 2, B_SHARD // 2).transpose(0, 2, 1, 3)
        )  # (4, 2, 128, 512)
        in_maps.append({"xt": xtc, "c3": c3_arr})
    return nc, in_maps


def kernel(x: np.ndarray, coefficients: np.ndarray, grid: np.ndarray) -> np.ndarray:
    nc, in_maps = _prepare(x, coefficients, grid)
    res = run_bass_kernel_spmd(nc, in_maps, list(range(N_CORES)), trace=False)
    outs = []
    for core in range(N_CORES):
        oc = res.results[core]["out"]  # (4, 2, 128, 512) = (o-chunk, half, part, col)
        out_t = oc.transpose(0, 2, 1, 3).reshape(O_FEAT, B_SHARD)  # (512, 1024)
        outs.append(out_t)
    out_full = np.concatenate(outs, axis=1)  # (512, 8192)
    return np.ascontiguousarray(out_full.T).astype(np.float32)


# revision 7
# speedup vs baseline: 1.1314x; 1.1314x over previous
"""Trainium2 Bass kernel for the B-spline (KAN-style) layer.

Math: out[b,o] = sum_{i,k} B3_k(t(b,i)) * coeff[i,o,k], where t = tanh(x)
mapped to knot coordinates t = (tanh(x) - grid[0]) / h in (3, 8), and B3 are
cubic B-spline bases over uniform integer knots.

Key transformation: each cubic B-spline basis is an exact linear combination
of the truncated cubes L_m = relu(t-m)^3 and the raw cubes c_m = (t-m)^3,
m in {4,5,6,7} (divided-difference identity; R_m = (m-t)_+^3 = L_m - c_m is
eliminated so only TWO planes per knot are computed on device):
    a_m = 2.5*tanh(x) + (5.5 - m)     (DVE tensor_scalar)
    s_m = Square(a_m)                 (ACT engine, affine fused into the op)
    c_m = s_m * a_m                   (DVE tensor_mul)
    L_m = max(c_m, 0)                 (DVE tensor_scalar_max)
The 8x8 basis-change matrix (planes [L4,c4,L5,c5,L6,c6,L7,c7]) is folded into
the coefficient tensor on the host.

Then a dense fp16 matmul: out[o,b] = sum_{(i,plane)} C3[(i,plane),o] * rho[(i,plane),b]
with contraction K = 512*8 = 4096, run on the PE at full fp16 rate.

Schedule notes (from trace analysis):
  - the 256-matmul stream runs at the theoretical 216.7ns/MM once started, so
    the wins are starting it earlier and shortening the tail;
  - plane production is split ACT (tanh+squares) / DVE (affine+cube+relu) so
    neither engine gates the stream;
  - chunk 0 half 0 is produced at half-batch width (FD=512) to minimize
    first-matmul latency; everything else at full width (FD=1024), which also
    means pass 2 (second batch half) needs no further elementwise work;
  - xt and out are stored tile-contiguous in DRAM so DMAs aren't
    descriptor/stride-bound.

Sharding: data-parallel over batch (8192 -> 8 x 1024); coefficients replicated.
"""

from contextlib import ExitStack, nullcontext

import numpy as np

import concourse.bass as bass
import concourse.mybir as mybir
import concourse.tile as tile
from concourse.bass_utils import run_bass_kernel_spmd
from concourse.tile import add_dep_helper
from concourse.vector_clock import ScopedClock

F32 = mybir.dt.float32
F16 = mybir.dt.float16

N_CORES = 8
B_FULL = 8192
B_SHARD = B_FULL // N_CORES  # 1024
I_FEAT = 512
O_FEAT = 512
NPLANES = 8
NCHUNK = I_FEAT // 128  # 4
ALU = mybir.AluOpType
AF = mybir.ActivationFunctionType

N_WARMUP = 10

# ---------------------------------------------------------------------------
# Workaround for walrus "Too many sync wait commands" on the TileContext final
# Drain: spread the accumulated semaphore waits across single-wait nofuse NOPs
# on the sync engine, then emit a bare drain + the usual barrier/cleanup.
_MAXW = 1


def _patched_drain_and_barrier(self, tick_clock, wait_clock):
    nc = self.nc
    probe = nc.sync.nop(nofuse=True)
    wait_clock.add_sem_waits(probe.ins, ScopedClock({None: tick_clock.global_clock}))
    si = probe.ins.sync_info
    waits = list(si.on_wait) if si and si.on_wait else []
    if len(waits) > _MAXW:
        si.on_wait = waits[:_MAXW]
        rest = waits[_MAXW:]
        while rest:
            chunk, rest = rest[:_MAXW], rest[_MAXW:]
            n2 = nc.sync.nop(nofuse=True)
            s2 = n2.ins.sync_info
            if s2 is None:
                n2.ins.sync_info = mybir.SyncInfo(on_wait=chunk, on_update=[])
            else:
                s2.on_wait = chunk
    nc.sync.drain()
    nc.all_engine_barrier()
    assert self.sems is not None
    popped = nc._tile_sem_poison_stack.pop()
    assert popped is self._sem_poison
    nc.clear_and_free_semaphores(list(self.sems.allocated().values()))


tile.TileContext._drain_and_barrier = _patched_drain_and_barrier


def _split_all_waits(nc: bass.Bass) -> None:
    """This image's walrus rejects instructions carrying more than one sync
    wait. Hoist all but the last wait of each instruction onto fresh NoOps on
    the same engine immediately before it (in-order issue makes this
    equivalent, merely slightly stronger synchronization)."""
    cnt = 0
    for f in nc.m.functions:
        for bb in f.blocks:
            out = []
            changed = False
            for inst in bb.instructions:
                si = inst.sync_info
                waits = list(si.on_wait) if si and si.on_wait else []
                if len(waits) > 1:
                    changed = True
                    for w in waits[:-1]:
                        nop = mybir.InstNoOp(name=f"waitsplit-{cnt}", ins=[], outs=[])
                        cnt += 1
                        nop.engine = inst.engine
                        nop.sync_info = mybir.SyncInfo(on_wait=[w], on_update=[])
                        out.append(nop)
                    si.on_wait = [waits[-1]]
                out.append(inst)
            if changed:
                bb.instructions = out


# ---------------------------------------------------------------------------


def _build_nc(t_scale: float, t_bias: float) -> bass.Bass:
    """Build the per-core Bass program.

    Per-core I/O (all tile-contiguous in DRAM):
      xt : (4, 2, 128, 512) f16   x^T shard (chunk, batch-half, part, col)
      c3 : (4, 128, 4096) f16     folded coefficients [chunk, part, plane*512+o]
      out: (4, 2, 128, 512) f32   output (o-chunk, batch-half, part, col)
    """
    nc = bass.Bass()
    # Register the Square-activation bias constants (t_bias - m) as const APs
    # (the activation instruction requires per-partition bias tensors).
    for m in (4, 5, 6, 7):
        v = float(t_bias - m)
        # [128, 16] keeps every SBUF allocation a 64B multiple — a 32B-odd
        # allocation here shifts all later tiles to 32-mod-64 addresses,
        # which slows the PE moving-operand fetch by ~20%.
        tens = nc.alloc_sbuf_tensor(f"const-bias-{m}", [128, 16], F32)
        nc.gpsimd.memset(tens.ap(), v)
        nc.const_aps.aps[(F32, v)] = tens.ap()[:, :1]
    nc.all_engine_barrier()
    xt = nc.declare_dram_parameter("xt", [NCHUNK, 2, 128, B_SHARD // 2], F16,
                                   isOutput=False)
    c3 = nc.declare_dram_parameter(
        "c3", [NCHUNK, 128, NPLANES * O_FEAT], F16, isOutput=False
    )
    out = nc.declare_dram_parameter("out", [NCHUNK, 2, 128, B_SHARD // 2], F32,
                                    isOutput=True)

    with tile.TileContext(nc) as tc, ExitStack() as ctx:
        c3_pool = ctx.enter_context(tc.tile_pool(name="c3", bufs=1))
        xin_pool = ctx.enter_context(tc.tile_pool(name="xin", bufs=1))
        xn_pool = ctx.enter_context(tc.tile_pool(name="xn", bufs=1))
        sq_pool = ctx.enter_context(tc.tile_pool(name="sq", bufs=3))
        aff_pool = ctx.enter_context(tc.tile_pool(name="aff", bufs=3))
        rho_pool = ctx.enter_context(tc.tile_pool(name="rho", bufs=1))
        ps_pool = ctx.enter_context(
            tc.tile_pool(name="ps", bufs=1, space=bass.MemorySpace.PSUM)
        )
        ost_pool = ctx.enter_context(tc.tile_pool(name="ost", bufs=1))

        BHALF = B_SHARD // 2  # 512

        # Dummy 1-column activation with no deps: hoists the ~1.3us ACT
        # table load to kernel start, off the tanh critical path.
        dummy = xn_pool.tile([128, 32], F16, tag="dummy")
        nc.gpsimd.memset(dummy[:], 0.0)
        nc.scalar.activation(dummy[:, :1], dummy[:, :1], AF.Tanh)

        # c3 streams on the gpsimd HWDGE ring (idle engine) so triggers don't
        # delay the ACT queue (table load + tanh) or the sync queue (xt).
        # Chunk 0 is split so the first K-slice (512 cols) lands early.
        c3_sb = []
        for c in range(NCHUNK):
            ct = c3_pool.tile([128, NPLANES * O_FEAT], F16, tag=f"c3_{c}")
            if c == 0:
                nc.gpsimd.dma_start(ct[:, :O_FEAT], c3[0][:, :O_FEAT])
                nc.gpsimd.dma_start(ct[:, O_FEAT:], c3[0][:, O_FEAT:])
            else:
                nc.gpsimd.dma_start(ct[:], c3[c])
            c3_sb.append(ct)

        # PE warm-up: zero matmuls with no data deps at kernel start, so the
        # HAM clock-gate reaches 8/8 before the first real matmul.
        wz = c3_pool.tile([128, 512], F16, tag="warmz")
        nc.gpsimd.memset(wz[:], 0.0)

        # rho[h][c][r] planes interleaved [L4, c4, L5, c5, ...] (basis change
        # folded into c3 on the host). Chunk 0 is produced at half-batch
        # width; chunks 1-3 at full width (slices feed both matmul passes).
        rho = [[[None] * NPLANES for _ in range(NCHUNK)] for _ in range(2)]

        def produce_unit(xn_t, c, mi, m, fd_half, prio_ctx, prev_end):
            """Emit one m-unit (two planes) for chunk c. fd_half: 0/1 for a
            half-batch unit on that half, None for full-batch."""
            beta = t_bias - m
            if fd_half is None:
                shape = [128, 2 * BHALF]
            else:
                shape = [128, BHALF]
            with prio_ctx:
                sq = sq_pool.tile(shape, F16, tag="sqh" if fd_half is not None else "sqf")
                sq_i = nc.scalar.activation(
                    sq[:], xn_t[:], AF.Square, bias=float(beta), scale=t_scale
                )
                a = aff_pool.tile(shape, F16, tag="ah" if fd_half is not None else "af")
                a_i = nc.vector.tensor_scalar(
                    a[:], xn_t[:], t_scale, beta, ALU.mult, ALU.add
                )
                if prev_end is not None:
                    # Order-only edge: keep the scheduler from interleaving
                    # the next m's ops into the chain that gates the matmul
                    # stream.
                    add_dep_helper(a_i.ins, prev_end.ins, sync=False,
                                   reason="first-chain order")
                cc = rho_pool.tile(shape, F16, tag=f"rho_c{c}_{2 * mi + 1}_{fd_half}")
                nc.vector.tensor_mul(cc[:], sq[:], a[:])
                lp = rho_pool.tile(shape, F16, tag=f"rho_c{c}_{2 * mi}_{fd_half}")
                lp_i = nc.vector.tensor_scalar_max(lp[:], cc[:], 0.0)
            return lp, cc, lp_i

        # --- chunk 0, half 0: half-batch production, priority-pinned ---
        xt00 = xin_pool.tile([128, BHALF], F16, tag="xt00")
        nc.sync.dma_start(xt00[:], xt[0][0])
        xn00 = xn_pool.tile([128, BHALF], F16, tag="xn00")
        nc.scalar.activation(xn00[:], xt00[:], AF.Tanh)
        half_planes = [[None] * NPLANES for _ in range(2)]  # chunk0 per half
        prev = None
        for mi, m in enumerate((4, 5, 6, 7)):
            prio = tc.high_priority() if mi == 0 else nullcontext()
            lp, cc, prev = produce_unit(xn00, 0, mi, m, 0, prio,
                                        prev if mi == 1 else None)
            half_planes[0][2 * mi] = lp
            half_planes[0][2 * mi + 1] = cc

        # --- chunks 1-3: full-batch production ---
        for c in range(1, NCHUNK):
            xtc = xin_pool.tile([128, 2 * BHALF], F16, tag=f"xt{c}")
            nc.sync.dma_start(xtc[:, :BHALF], xt[c][0])
            nc.sync.dma_start(xtc[:, BHALF:], xt[c][1])
            xnc = xn_pool.tile([128, 2 * BHALF], F16, tag=f"xn{c}")
            nc.scalar.activation(xnc[:], xtc[:], AF.Tanh)
            for mi, m in enumerate((4, 5, 6, 7)):
                lp, cc, _ = produce_unit(xnc, c, mi, m, None, nullcontext(), None)
                for h in range(2):
                    rho[h][c][2 * mi] = lp[:, h * BHALF:(h + 1) * BHALF]
                    rho[h][c][2 * mi + 1] = cc[:, h * BHALF:(h + 1) * BHALF]

        # --- chunk 0, half 1: half-batch production (needed only by pass 2) ---
        xt01 = xin_pool.tile([128, BHALF], F16, tag="xt01")
        nc.sync.dma_start(xt01[:], xt[0][1])
        xn01 = xn_pool.tile([128, BHALF], F16, tag="xn01")
        nc.scalar.activation(xn01[:], xt01[:], AF.Tanh)
        for mi, m in enumerate((4, 5, 6, 7)):
            lp, cc, _ = produce_unit(xn01, 0, mi, m, 1, nullcontext(), None)
            half_planes[1][2 * mi] = lp
            half_planes[1][2 * mi + 1] = cc
        for h in range(2):
            for r in range(NPLANES):
                rho[h][0][r] = half_planes[h][r][:]

        # Dense matmul: 8 PSUM tiles (o_chunk x b_half) accumulated over all
        # 32 (chunk, plane) K-slices. Pass h=0 is K-major (dense PE stream
        # consuming planes in production order); its PSUM eviction + output
        # DMA overlap pass h=1. Pass h=1 is o-major so each o-tile's
        # eviction + DMA trickle out during the remaining matmuls.
        ps = [
            [
                ps_pool.tile([128, 512], F32, tag=f"ps{o}_{h}", name=f"ps{o}_{h}")
                for h in range(2)
            ]
            for o in range(NCHUNK)
        ]
        NK = NCHUNK * NPLANES

        for _ in range(N_WARMUP):
            nc.tensor.matmul(
                ps[0][0][:64, :], wz[:, :64], wz[:], start=True, stop=True
            )

        def emit_copy_out(o, h):
            # Copies alternate ACT/DVE; output DMA triggers alternate the
            # sync/gpsimd HWDGE rings so the tail's trigger issue
            # parallelizes instead of queueing.
            ot = ost_pool.tile([128, 512], F32, tag=f"ot{o}_{h}", name=f"ot{o}_{h}")
            if o % 2 == 0:
                nc.scalar.activation(ot[:], ps[o][h][:], AF.Copy)
            else:
                nc.vector.tensor_copy(ot[:], ps[o][h][:])
            eng = nc.sync if o % 2 == 0 else nc.gpsimd
            eng.dma_start(out[o][h], ot[:])

        for kk in range(NK):
            c, r = divmod(kk, NPLANES)
            rt = rho[0][c][r]
            for o in range(4):
                lhsT = c3_sb[c][:, r * O_FEAT + o * 128 : r * O_FEAT + (o + 1) * 128]
                nc.tensor.matmul(
                    ps[o][0][:], lhsT, rt, start=(kk == 0), stop=(kk == NK - 1)
                )
        for o in range(4):
            emit_copy_out(o, 0)
        KTAIL = NK - 4
        for kk in range(KTAIL):
            c, r = divmod(kk, NPLANES)
            rt = rho[1][c][r]
            for o in range(4):
                lhsT = c3_sb[c][:, r * O_FEAT + o * 128 : r * O_FEAT + (o + 1) * 128]
                nc.tensor.matmul(
                    ps[o][1][:], lhsT, rt, start=(kk == 0), stop=False
                )
        for o in range(4):
            for kk in range(KTAIL, NK):
                c, r = divmod(kk, NPLANES)
                lhsT = c3_sb[c][:, r * O_FEAT + o * 128 : r * O_FEAT + (o + 1) * 128]
                nc.tensor.matmul(
                    ps[o][1][:],
                    lhsT,
                    rho[1][c][r],
                    start=False,
                    stop=(kk == NK - 1),
                )
            emit_copy_out(o, 1)
    _split_all_waits(nc)
    return nc


# Basis-change: with L_m = (t-m)_+^3, R_m = (m-t)_+^3, c_m = (t-m)^3 = L_m - R_m,
# B3[j](t) = sum_m [WL[j,m]*L_m + WR[j,m]*R_m]/6
#          = sum_m [(WL+WR)[j,m]*L_m - WR[j,m]*c_m]/6
# (binomial divided-difference weights).
_WL = np.array(
    [
        [0, 0, 0, 0],
        [0, 0, 0, 0],
        [0, 0, 0, 0],
        [0, 0, 0, 0],
        [1, -4, 6, -4],
        [0, 1, -4, 6],
        [0, 0, 1, -4],
        [0, 0, 0, 1],
    ],
    dtype=np.float64,
)
_WR = np.array(
    [
        [1, 0, 0, 0],
        [-4, 1, 0, 0],
        [6, -4, 1, 0],
        [-4, 6, -4, 1],
        [0, 0, 0, 0],
        [0, 0, 0, 0],
        [0, 0, 0, 0],
        [0, 0, 0, 0],
    ],
    dtype=np.float64,
)

_nc_cache: dict = {}


def _prepare(x: np.ndarray, coefficients: np.ndarray, grid: np.ndarray):
    x = np.asarray(x, dtype=np.float32)
    coefficients = np.asarray(coefficients, dtype=np.float32)
    grid = np.asarray(grid, dtype=np.float32)

    # Knot-coordinate transform t = (tanh(x) - grid[0]) / h (uniform grid).
    h = float(grid[-1] - grid[0]) / (len(grid) - 1)
    t_scale = 1.0 / h
    t_bias = -float(grid[0]) / h  # t = t_scale * xn + t_bias; here 2.5, 5.5

    key = (round(t_scale, 9), round(t_bias, 9))
    if key not in _nc_cache:
        _nc_cache[key] = _build_nc(t_scale, t_bias)
    nc = _nc_cache[key]

    # Host-side coefficient fold: C3[i, r, o] = sum_j coeff[i,o,j] * W2[j,r],
    # planes interleaved [L4, c4, L5, c5, ...].
    w2 = np.zeros((8, 8), dtype=np.float64)
    for mi in range(4):
        w2[:, 2 * mi] = (_WL[:, mi] + _WR[:, mi]) / 6.0
        w2[:, 2 * mi + 1] = -_WR[:, mi] / 6.0
    c3f = np.einsum("ioj,jr->iro", coefficients.astype(np.float64), w2)
    c3_arr = np.ascontiguousarray(
        c3f.reshape(NCHUNK, 128, NPLANES, O_FEAT)
        .reshape(NCHUNK, 128, NPLANES * O_FEAT)
        .astype(np.float16)
    )

    # xt tile-contiguous: (chunk, half, part, col) per core.
    xt = x.T.astype(np.float16)  # (512, 8192)
    in_maps = []
    for core in range(N_CORES):
        xs = xt[:, core * B_SHARD : (core + 1) * B_SHARD]  # (512, 1024)
        xtc = np.ascontiguousarray(
            xs.reshape(NCHUNK, 128, 2, B_SHARD // 2).transpose(0, 2, 1, 3)
        )  # (4, 2, 128, 512)
        in_maps.append({"xt": xtc, "c3": c3_arr})
    return nc, in_maps


def kernel(x: np.ndarray, coefficients: np.ndarray, grid: np.ndarray) -> np.ndarray:
    nc, in_maps = _prepare(x, coefficients, grid)
    res = run_bass_kernel_spmd(nc, in_maps, list(range(N_CORES)), trace=False)
    outs = []
    for core in range(N_CORES):
        oc = res.results[core]["out"]  # (4, 2, 128, 512) = (o-chunk, half, part, col)
        out_t = oc.transpose(0, 2, 1, 3).reshape(O_FEAT, B_SHARD)  # (512, 1024)
        outs.append(out_t)
    out_full = np.concatenate(outs, axis=1)  # (512, 8192)
    return np.ascontiguousarray(out_full.T).astype(np.float32)


# revision 10
# speedup vs baseline: 1.2211x; 1.0793x over previous
"""Trainium2 Bass kernel for the B-spline (KAN-style) layer.

Math: out[b,o] = sum_{i,k} B3_k(t(b,i)) * coeff[i,o,k], where t = tanh(x)
mapped to knot coordinates t = (tanh(x) - grid[0]) / h in (3, 8), and B3 are
cubic B-spline bases over uniform integer knots.

Key transformation: each cubic B-spline basis is an exact linear combination
of the truncated cubes L_m = relu(t-m)^3 and the raw cubes c_m = (t-m)^3,
m in {4,5,6,7} (divided-difference identity; R_m = (m-t)_+^3 = L_m - c_m is
eliminated so only TWO planes per knot are computed on device):
    a_m = 2.5*tanh(x) + (5.5 - m)     (DVE tensor_scalar)
    s_m = Square(a_m)                 (ACT engine, affine fused into the op)
    c_m = s_m * a_m                   (DVE tensor_mul)
    L_m = max(c_m, 0)                 (DVE tensor_scalar_max)
The 8x8 basis-change matrix (planes [L4,c4,L5,c5,L6,c6,L7,c7]) is folded into
the coefficient tensor on the host.

Then a dense fp16 matmul: out[o,b] = sum_{(i,plane)} C3[(i,plane),o] * rho[(i,plane),b]
with contraction K = 512*8 = 4096, run on the PE at full fp16 rate.

Schedule notes (from trace analysis):
  - the 256-matmul stream runs at the theoretical 216.7ns/MM once started, so
    the wins are starting it earlier and shortening the tail;
  - plane production is split ACT (tanh+squares) / DVE (affine+cube+relu) so
    neither engine gates the stream;
  - chunk 0 half 0 is produced at half-batch width (FD=512) to minimize
    first-matmul latency; everything else at full width (FD=1024), which also
    means pass 2 (second batch half) needs no further elementwise work;
  - xt and out are stored tile-contiguous in DRAM so DMAs aren't
    descriptor/stride-bound.

Sharding: data-parallel over batch (8192 -> 8 x 1024); coefficients replicated.
"""

from contextlib import ExitStack, nullcontext

import numpy as np

import concourse.bass as bass
import concourse.mybir as mybir
import concourse.tile as tile
from concourse.bass_utils import run_bass_kernel_spmd
from concourse.tile import add_dep_helper
from concourse.vector_clock import ScopedClock

F32 = mybir.dt.float32
F16 = mybir.dt.float16

N_CORES = 8
B_FULL = 8192
B_SHARD = B_FULL // N_CORES  # 1024
I_FEAT = 512
O_FEAT = 512
NPLANES = 8
NCHUNK = I_FEAT // 128  # 4
ALU = mybir.AluOpType
AF = mybir.ActivationFunctionType

N_WARMUP = 10

# ---------------------------------------------------------------------------
# Workaround for walrus "Too many sync wait commands" on the TileContext final
# Drain: spread the accumulated semaphore waits across single-wait nofuse NOPs
# on the sync engine, then emit a bare drain + the usual barrier/cleanup.
_MAXW = 1


def _patched_drain_and_barrier(self, tick_clock, wait_clock):
    nc = self.nc
    probe = nc.sync.nop(nofuse=True)
    wait_clock.add_sem_waits(probe.ins, ScopedClock({None: tick_clock.global_clock}))
    si = probe.ins.sync_info
    waits = list(si.on_wait) if si and si.on_wait else []
    if len(waits) > _MAXW:
        si.on_wait = waits[:_MAXW]
        rest = waits[_MAXW:]
        while rest:
            chunk, rest = rest[:_MAXW], rest[_MAXW:]
            n2 = nc.sync.nop(nofuse=True)
            s2 = n2.ins.sync_info
            if s2 is None:
                n2.ins.sync_info = mybir.SyncInfo(on_wait=chunk, on_update=[])
            else:
                s2.on_wait = chunk
    nc.sync.drain()
    nc.all_engine_barrier()
    assert self.sems is not None
    popped = nc._tile_sem_poison_stack.pop()
    assert popped is self._sem_poison
    nc.clear_and_free_semaphores(list(self.sems.allocated().values()))


tile.TileContext._drain_and_barrier = _patched_drain_and_barrier


def _split_all_waits(nc: bass.Bass) -> None:
    """This image's walrus rejects instructions carrying more than one sync
    wait. Hoist all but the last wait of each instruction onto fresh NoOps on
    the same engine immediately before it (in-order issue makes this
    equivalent, merely slightly stronger synchronization)."""
    cnt = 0
    for f in nc.m.functions:
        for bb in f.blocks:
            out = []
            changed = False
            for inst in bb.instructions:
                si = inst.sync_info
                waits = list(si.on_wait) if si and si.on_wait else []
                if len(waits) > 1:
                    changed = True
                    for w in waits[:-1]:
                        nop = mybir.InstNoOp(name=f"waitsplit-{cnt}", ins=[], outs=[])
                        cnt += 1
                        nop.engine = inst.engine
                        nop.sync_info = mybir.SyncInfo(on_wait=[w], on_update=[])
                        out.append(nop)
                    si.on_wait = [waits[-1]]
                out.append(inst)
            if changed:
                bb.instructions = out


# ---------------------------------------------------------------------------


def _build_nc(t_scale: float, t_bias: float) -> bass.Bass:
    """Build the per-core Bass program.

    Per-core I/O (all tile-contiguous in DRAM):
      xt : (4, 2, 128, 512) f16   x^T shard (chunk, batch-half, part, col)
      c3 : (4, 128, 4096) f16     folded coefficients [chunk, part, plane*512+o]
      out: (4, 2, 128, 512) f32   output (o-chunk, batch-half, part, col)
    """
    nc = bass.Bass()
    # Register the Square-activation bias constants (t_bias - m) as const APs
    # (the activation instruction requires per-partition bias tensors).
    for m in (4, 5, 6, 7):
        v = float(t_bias - m)
        # [128, 16] keeps every SBUF allocation a 64B multiple — a 32B-odd
        # allocation here shifts all later tiles to 32-mod-64 addresses,
        # which slows the PE moving-operand fetch by ~20%.
        tens = nc.alloc_sbuf_tensor(f"const-bias-{m}", [128, 16], F32)
        nc.gpsimd.memset(tens.ap(), v)
        nc.const_aps.aps[(F32, v)] = tens.ap()[:, :1]
    nc.all_engine_barrier()
    xt = nc.declare_dram_parameter("xt", [NCHUNK, 2, 128, B_SHARD // 2], F16,
                                   isOutput=False)
    c3 = nc.declare_dram_parameter(
        "c3", [NCHUNK, 128, NPLANES * O_FEAT], F16, isOutput=False
    )
    out = nc.declare_dram_parameter("out", [NCHUNK, 2, 128, B_SHARD // 2], F32,
                                    isOutput=True)

    with tile.TileContext(nc) as tc, ExitStack() as ctx:
        c3_pool = ctx.enter_context(tc.tile_pool(name="c3", bufs=1))
        xin_pool = ctx.enter_context(tc.tile_pool(name="xin", bufs=1))
        xn_pool = ctx.enter_context(tc.tile_pool(name="xn", bufs=1))
        sq_pool = ctx.enter_context(tc.tile_pool(name="sq", bufs=3))
        aff_pool = ctx.enter_context(tc.tile_pool(name="aff", bufs=3))
        rho_pool = ctx.enter_context(tc.tile_pool(name="rho", bufs=1))
        ps_pool = ctx.enter_context(
            tc.tile_pool(name="ps", bufs=1, space=bass.MemorySpace.PSUM)
        )
        ost_pool = ctx.enter_context(tc.tile_pool(name="ost", bufs=1))

        BHALF = B_SHARD // 2  # 512

        # Dummy 1-column activation with no deps: hoists the ~1.3us ACT
        # table load to kernel start, off the tanh critical path.
        dummy = xn_pool.tile([128, 32], F16, tag="dummy")
        nc.gpsimd.memset(dummy[:], 0.0)
        nc.scalar.activation(dummy[:, :1], dummy[:, :1], AF.Tanh)

        # PE warm-up source: memset FIRST on gpsimd so the warm-up matmuls
        # start as early as possible (the c3 triggers below would otherwise
        # push them out by ~3us).
        wz = c3_pool.tile([128, 512], F16, tag="warmz")
        nc.gpsimd.memset(wz[:], 0.0)

        # c3 streams on the gpsimd HWDGE ring (idle engine) so triggers don't
        # delay the ACT queue (table load + tanh) or the sync queue (xt).
        c3_sb = []
        for c in range(NCHUNK):
            ct = c3_pool.tile([128, NPLANES * O_FEAT], F16, tag=f"c3_{c}")
            nc.gpsimd.dma_start(ct[:], c3[c])
            c3_sb.append(ct)

        # rho[h][c][r] planes interleaved [L4, c4, L5, c5, ...] (basis change
        # folded into c3 on the host). Chunk 0 is produced at half-batch
        # width; chunks 1-3 at full width (slices feed both matmul passes).
        rho = [[[None] * NPLANES for _ in range(NCHUNK)] for _ in range(2)]

        # The ACT queue must execute tanh/squares strictly in production
        # order — the list scheduler otherwise hoists a later chunk's tanh
        # (gated on its xt DMA) ahead of the current chunk's squares,
        # starving the matmul stream for several us.
        act_chain = [None]

        def chain_act(inst):
            if act_chain[0] is not None:
                add_dep_helper(inst.ins, act_chain[0].ins, sync=False,
                               reason="ACT production order")
            act_chain[0] = inst
            return inst

        def produce_unit(xn_t, c, mi, m, fd_half, prio_ctx, prev_end):
            """Emit one m-unit (two planes) for chunk c. fd_half: 0/1 for a
            half-batch unit on that half, None for full-batch."""
            beta = t_bias - m
            if fd_half is None:
                shape = [128, 2 * BHALF]
            else:
                shape = [128, BHALF]
            with prio_ctx:
                sq = sq_pool.tile(shape, F16, tag="sqh" if fd_half is not None else "sqf")
                chain_act(nc.scalar.activation(
                    sq[:], xn_t[:], AF.Square, bias=float(beta), scale=t_scale
                ))
                a = aff_pool.tile(shape, F16, tag="ah" if fd_half is not None else "af")
                a_i = nc.vector.tensor_scalar(
                    a[:], xn_t[:], t_scale, beta, ALU.mult, ALU.add
                )
                if prev_end is not None:
                    # Order-only edge: keep the scheduler from interleaving
                    # the next m's ops into the chain that gates the matmul
                    # stream.
                    add_dep_helper(a_i.ins, prev_end.ins, sync=False,
                                   reason="first-chain order")
                cc = rho_pool.tile(shape, F16, tag=f"rho_c{c}_{2 * mi + 1}_{fd_half}")
                nc.vector.tensor_mul(cc[:], sq[:], a[:])
                lp = rho_pool.tile(shape, F16, tag=f"rho_c{c}_{2 * mi}_{fd_half}")
                lp_i = nc.vector.tensor_scalar_max(lp[:], cc[:], 0.0)
            return lp, cc, lp_i

        # xt DMAs are spread across the sync/vector/scalar HWDGE rings so the
        # transfers overlap instead of queueing behind one ring.
        xt_ring = {0: nc.sync, 1: nc.sync, 2: nc.scalar, 3: nc.gpsimd}

        # --- chunk 0, half 0: half-batch production, priority-pinned ---
        xt00 = xin_pool.tile([128, BHALF], F16, tag="xt00")
        nc.sync.dma_start(xt00[:], xt[0][0])
        xn00 = xn_pool.tile([128, BHALF], F16, tag="xn00")
        chain_act(nc.scalar.activation(xn00[:], xt00[:], AF.Tanh))
        half_planes = [[None] * NPLANES for _ in range(2)]  # chunk0 per half
        prev = None
        for mi, m in enumerate((4, 5, 6, 7)):
            prio = tc.high_priority() if mi == 0 else nullcontext()
            lp, cc, prev = produce_unit(xn00, 0, mi, m, 0, prio,
                                        prev if mi == 1 else None)
            half_planes[0][2 * mi] = lp
            half_planes[0][2 * mi + 1] = cc

        # --- chunks 1-3: full-batch production ---
        for c in range(1, NCHUNK):
            xtc = xin_pool.tile([128, 2 * BHALF], F16, tag=f"xt{c}")
            ring = xt_ring[c]
            ring.dma_start(xtc[:, :BHALF], xt[c][0])
            ring.dma_start(xtc[:, BHALF:], xt[c][1])
            xnc = xn_pool.tile([128, 2 * BHALF], F16, tag=f"xn{c}")
            chain_act(nc.scalar.activation(xnc[:], xtc[:], AF.Tanh))
            for mi, m in enumerate((4, 5, 6, 7)):
                lp, cc, _ = produce_unit(xnc, c, mi, m, None, nullcontext(), None)
                for h in range(2):
                    rho[h][c][2 * mi] = lp[:, h * BHALF:(h + 1) * BHALF]
                    rho[h][c][2 * mi + 1] = cc[:, h * BHALF:(h + 1) * BHALF]

        # --- chunk 0, half 1: half-batch production (needed only by pass 2) ---
        xt01 = xin_pool.tile([128, BHALF], F16, tag="xt01")
        nc.sync.dma_start(xt01[:], xt[0][1])
        xn01 = xn_pool.tile([128, BHALF], F16, tag="xn01")
        chain_act(nc.scalar.activation(xn01[:], xt01[:], AF.Tanh))
        for mi, m in enumerate((4, 5, 6, 7)):
            lp, cc, _ = produce_unit(xn01, 0, mi, m, 1, nullcontext(), None)
            half_planes[1][2 * mi] = lp
            half_planes[1][2 * mi + 1] = cc
        for h in range(2):
            for r in range(NPLANES):
                rho[h][0][r] = half_planes[h][r][:]

        # Dense matmul: 8 PSUM tiles (o_chunk x b_half) accumulated over all
        # 32 (chunk, plane) K-slices. Pass h=0 is K-major (dense PE stream
        # consuming planes in production order); its PSUM eviction + output
        # DMA overlap pass h=1. Pass h=1 is o-major so each o-tile's
        # eviction + DMA trickle out during the remaining matmuls.
        ps = [
            [
                ps_pool.tile([128, 512], F32, tag=f"ps{o}_{h}", name=f"ps{o}_{h}")
                for h in range(2)
            ]
            for o in range(NCHUNK)
        ]
        NK = NCHUNK * NPLANES

        for _ in range(N_WARMUP):
            nc.tensor.matmul(
                ps[0][0][:64, :], wz[:, :64], wz[:], start=True, stop=True
            )

        def emit_copy_out(o, h):
            # Copies alternate ACT/DVE; output DMA triggers alternate the
            # sync/gpsimd HWDGE rings so the tail's trigger issue
            # parallelizes instead of queueing.
            ot = ost_pool.tile([128, 512], F32, tag=f"ot{o}_{h}", name=f"ot{o}_{h}")
            if o % 2 == 0:
                nc.scalar.activation(ot[:], ps[o][h][:], AF.Copy)
            else:
                nc.vector.tensor_copy(ot[:], ps[o][h][:])
            eng = nc.sync if o % 2 == 0 else nc.gpsimd
            eng.dma_start(out[o][h], ot[:])

        for kk in range(NK):
            c, r = divmod(kk, NPLANES)
            rt = rho[0][c][r]
            for o in range(4):
                lhsT = c3_sb[c][:, r * O_FEAT + o * 128 : r * O_FEAT + (o + 1) * 128]
                nc.tensor.matmul(
                    ps[o][0][:], lhsT, rt, start=(kk == 0), stop=(kk == NK - 1)
                )
        for o in range(4):
            emit_copy_out(o, 0)
        KTAIL = NK - 4
        for kk in range(KTAIL):
            c, r = divmod(kk, NPLANES)
            rt = rho[1][c][r]
            for o in range(4):
                lhsT = c3_sb[c][:, r * O_FEAT + o * 128 : r * O_FEAT + (o + 1) * 128]
                nc.tensor.matmul(
                    ps[o][1][:], lhsT, rt, start=(kk == 0), stop=False
                )
        for o in range(4):
            for kk in range(KTAIL, NK):
                c, r = divmod(kk, NPLANES)
                lhsT = c3_sb[c][:, r * O_FEAT + o * 128 : r * O_FEAT + (o + 1) * 128]
                nc.tensor.matmul(
                    ps[o][1][:],
                    lhsT,
                    rho[1][c][r],
                    start=False,
                    stop=(kk == NK - 1),
                )
            emit_copy_out(o, 1)
    _split_all_waits(nc)
    return nc


# Basis-change: with L_m = (t-m)_+^3, R_m = (m-t)_+^3, c_m = (t-m)^3 = L_m - R_m,
# B3[j](t) = sum_m [WL[j,m]*L_m + WR[j,m]*R_m]/6
#          = sum_m [(WL+WR)[j,m]*L_m - WR[j,m]*c_m]/6
# (binomial divided-difference weights).
_WL = np.array(
    [
        [0, 0, 0, 0],
        [0, 0, 0, 0],
        [0, 0, 0, 0],
        [0, 0, 0, 0],
        [1, -4, 6, -4],
        [0, 1, -4, 6],
        [0, 0, 1, -4],
        [0, 0, 0, 1],
    ],
    dtype=np.float64,
)
_WR = np.array(
    [
        [1, 0, 0, 0],
        [-4, 1, 0, 0],
        [6, -4, 1, 0],
        [-4, 6, -4, 1],
        [0, 0, 0, 0],
        [0, 0, 0, 0],
        [0, 0, 0, 0],
        [0, 0, 0, 0],
    ],
    dtype=np.float64,
)

_nc_cache: dict = {}


def _prepare(x: np.ndarray, coefficients: np.ndarray, grid: np.ndarray):
    x = np.asarray(x, dtype=np.float32)
    coefficients = np.asarray(coefficients, dtype=np.float32)
    grid = np.asarray(grid, dtype=np.float32)

    # Knot-coordinate transform t = (tanh(x) - grid[0]) / h (uniform grid).
    h = float(grid[-1] - grid[0]) / (len(grid) - 1)
    t_scale = 1.0 / h
    t_bias = -float(grid[0]) / h  # t = t_scale * xn + t_bias; here 2.5, 5.5

    key = (round(t_scale, 9), round(t_bias, 9))
    if key not in _nc_cache:
        _nc_cache[key] = _build_nc(t_scale, t_bias)
    nc = _nc_cache[key]

    # Host-side coefficient fold: C3[i, r, o] = sum_j coeff[i,o,j] * W2[j,r],
    # planes interleaved [L4, c4, L5, c5, ...].
    w2 = np.zeros((8, 8), dtype=np.float64)
    for mi in range(4):
        w2[:, 2 * mi] = (_WL[:, mi] + _WR[:, mi]) / 6.0
        w2[:, 2 * mi + 1] = -_WR[:, mi] / 6.0
    c3f = np.einsum("ioj,jr->iro", coefficients.astype(np.float64), w2)
    c3_arr = np.ascontiguousarray(
        c3f.reshape(NCHUNK, 128, NPLANES, O_FEAT)
        .reshape(NCHUNK, 128, NPLANES * O_FEAT)
        .astype(np.float16)
    )

    # xt tile-contiguous: (chunk, half, part, col) per core.
    xt = x.T.astype(np.float16)  # (512, 8192)
    in_maps = []
    for core in range(N_CORES):
        xs = xt[:, core * B_SHARD : (core + 1) * B_SHARD]  # (512, 1024)
        xtc = np.ascontiguousarray(
            xs.reshape(NCHUNK, 128, 2, B_SHARD // 2).transpose(0, 2, 1, 3)
        )  # (4, 2, 128, 512)
        in_maps.append({"xt": xtc, "c3": c3_arr})
    return nc, in_maps


def kernel(x: np.ndarray, coefficients: np.ndarray, grid: np.ndarray) -> np.ndarray:
    nc, in_maps = _prepare(x, coefficients, grid)
    res = run_bass_kernel_spmd(nc, in_maps, list(range(N_CORES)), trace=False)
    outs = []
    for core in range(N_CORES):
        oc = res.results[core]["out"]  # (4, 2, 128, 512) = (o-chunk, half, part, col)
        out_t = oc.transpose(0, 2, 1, 3).reshape(O_FEAT, B_SHARD)  # (512, 1024)
        outs.append(out_t)
    out_full = np.concatenate(outs, axis=1)  # (512, 8192)
    return np.ascontiguousarray(out_full.T).astype(np.float32)


# revision 14
# speedup vs baseline: 1.2417x; 1.0169x over previous
"""Trainium2 Bass kernel for the B-spline (KAN-style) layer.

Math: out[b,o] = sum_{i,k} B3_k(t(b,i)) * coeff[i,o,k], where t = tanh(x)
mapped to knot coordinates t = (tanh(x) - grid[0]) / h in (3, 8), and B3 are
cubic B-spline bases over uniform integer knots.

Key transformation: each cubic B-spline basis is an exact linear combination
of the truncated cubes L_m = relu(t-m)^3 and the raw cubes c_m = (t-m)^3,
m in {4,5,6,7} (divided-difference identity; R_m = (m-t)_+^3 = L_m - c_m is
eliminated so only TWO planes per knot are computed on device):
    a_m = 2.5*tanh(x) + (5.5 - m)     (DVE tensor_scalar)
    s_m = Square(a_m)                 (ACT engine, affine fused into the op)
    c_m = s_m * a_m                   (DVE tensor_mul)
    L_m = max(c_m, 0)                 (DVE tensor_scalar_max)
The 8x8 basis-change matrix (planes [L4,c4,L5,c5,L6,c6,L7,c7]) is folded into
the coefficient tensor on the host.

Then a dense fp16 matmul: out[o,b] = sum_{(i,plane)} C3[(i,plane),o] * rho[(i,plane),b]
with contraction K = 512*8 = 4096, run on the PE at full fp16 rate.

Schedule notes (from trace analysis):
  - the 256-matmul stream runs at the theoretical 216.7ns/MM once started, so
    the wins are starting it earlier and shortening the tail;
  - plane production is split ACT (tanh+squares) / DVE (affine+cube+relu) so
    neither engine gates the stream;
  - chunk 0 half 0 is produced at half-batch width (FD=512) to minimize
    first-matmul latency; everything else at full width (FD=1024), which also
    means pass 2 (second batch half) needs no further elementwise work;
  - xt and out are stored tile-contiguous in DRAM so DMAs aren't
    descriptor/stride-bound.

Sharding: data-parallel over batch (8192 -> 8 x 1024); coefficients replicated.
"""

from contextlib import ExitStack, nullcontext

import numpy as np

import concourse.bass as bass
import concourse.mybir as mybir
import concourse.tile as tile
from concourse.bass_utils import run_bass_kernel_spmd
from concourse.tile import add_dep_helper
from concourse.vector_clock import ScopedClock

F32 = mybir.dt.float32
F16 = mybir.dt.float16

N_CORES = 8
B_FULL = 8192
B_SHARD = B_FULL // N_CORES  # 1024
I_FEAT = 512
O_FEAT = 512
NPLANES = 8
NCHUNK = I_FEAT // 128  # 4
ALU = mybir.AluOpType
AF = mybir.ActivationFunctionType

N_WARMUP = 6

# ---------------------------------------------------------------------------
# Workaround for walrus "Too many sync wait commands" on the TileContext final
# Drain: spread the accumulated semaphore waits across single-wait nofuse NOPs
# on the sync engine, then emit a bare drain + the usual barrier/cleanup.
_MAXW = 1


def _patched_drain_and_barrier(self, tick_clock, wait_clock):
    nc = self.nc
    probe = nc.sync.nop(nofuse=True)
    wait_clock.add_sem_waits(probe.ins, ScopedClock({None: tick_clock.global_clock}))
    si = probe.ins.sync_info
    waits = list(si.on_wait) if si and si.on_wait else []
    if len(waits) > _MAXW:
        si.on_wait = waits[:_MAXW]
        rest = waits[_MAXW:]
        while rest:
            chunk, rest = rest[:_MAXW], rest[_MAXW:]
            n2 = nc.sync.nop(nofuse=True)
            s2 = n2.ins.sync_info
            if s2 is None:
                n2.ins.sync_info = mybir.SyncInfo(on_wait=chunk, on_update=[])
            else:
                s2.on_wait = chunk
    nc.sync.drain()
    nc.all_engine_barrier()
    assert self.sems is not None
    popped = nc._tile_sem_poison_stack.pop()
    assert popped is self._sem_poison
    nc.clear_and_free_semaphores(list(self.sems.allocated().values()))


tile.TileContext._drain_and_barrier = _patched_drain_and_barrier


def _split_all_waits(nc: bass.Bass) -> None:
    """This image's walrus rejects instructions carrying more than one sync
    wait. Hoist all but the last wait of each instruction onto fresh NoOps on
    the same engine immediately before it (in-order issue makes this
    equivalent, merely slightly stronger synchronization)."""
    cnt = 0
    for f in nc.m.functions:
        for bb in f.blocks:
            out = []
            changed = False
            for inst in bb.instructions:
                si = inst.sync_info
                waits = list(si.on_wait) if si and si.on_wait else []
                if len(waits) > 1:
                    changed = True
                    for w in waits[:-1]:
                        nop = mybir.InstNoOp(name=f"waitsplit-{cnt}", ins=[], outs=[])
                        cnt += 1
                        nop.engine = inst.engine
                        nop.sync_info = mybir.SyncInfo(on_wait=[w], on_update=[])
                        out.append(nop)
                    si.on_wait = [waits[-1]]
                out.append(inst)
            if changed:
                bb.instructions = out


# ---------------------------------------------------------------------------


def _build_nc(t_scale: float, t_bias: float) -> bass.Bass:
    """Build the per-core Bass program.

    Per-core I/O (all tile-contiguous in DRAM):
      xt : (4, 2, 128, 512) f16   x^T shard (chunk, batch-half, part, col)
      c3 : (4, 128, 4096) f16     folded coefficients [chunk, part, plane*512+o]
      out: (4, 2, 128, 512) f32   output (o-chunk, batch-half, part, col)
    """
    nc = bass.Bass()
    xt = nc.declare_dram_parameter("xt", [NCHUNK, 2, 128, B_SHARD // 2], F16,
                                   isOutput=False)
    c3 = nc.declare_dram_parameter(
        "c3", [NCHUNK, 128, NPLANES * O_FEAT], F16, isOutput=False
    )
    out = nc.declare_dram_parameter("out", [NCHUNK, 2, 128, B_SHARD // 2], F32,
                                    isOutput=True)

    with tile.TileContext(nc) as tc, ExitStack() as ctx:
        c3_pool = ctx.enter_context(tc.tile_pool(name="c3", bufs=1))
        xin_pool = ctx.enter_context(tc.tile_pool(name="xin", bufs=1))
        xn_pool = ctx.enter_context(tc.tile_pool(name="xn", bufs=1))
        sq_pool = ctx.enter_context(tc.tile_pool(name="sq", bufs=3))
        aff_pool = ctx.enter_context(tc.tile_pool(name="aff", bufs=3))
        rho_pool = ctx.enter_context(tc.tile_pool(name="rho", bufs=1))
        ps_pool = ctx.enter_context(
            tc.tile_pool(name="ps", bufs=1, space=bass.MemorySpace.PSUM)
        )
        ost_pool = ctx.enter_context(tc.tile_pool(name="ost", bufs=1))

        BHALF = B_SHARD // 2  # 512

        # Dummy 1-column activation with no deps: hoists the ~1.3us ACT
        # table load to kernel start, off the tanh critical path.
        dummy = xn_pool.tile([128, 32], F16, tag="dummy")
        nc.gpsimd.memset(dummy[:], 0.0)
        nc.scalar.activation(dummy[:, :1], dummy[:, :1], AF.Tanh)

        # PE warm-up source: memset FIRST on gpsimd so the warm-up matmuls
        # start as early as possible (the c3 triggers below would otherwise
        # push them out by ~3us).
        wz = c3_pool.tile([128, 512], F16, tag="warmz")
        nc.gpsimd.memset(wz[:], 0.0)

        # c3 streams on the gpsimd HWDGE ring (idle engine) so triggers don't
        # delay the ACT queue (table load + tanh) or the sync queue (xt).
        c3_sb = []
        for c in range(NCHUNK):
            ct = c3_pool.tile([128, NPLANES * O_FEAT], F16, tag=f"c3_{c}")
            nc.gpsimd.dma_start(ct[:], c3[c])
            c3_sb.append(ct)

        # rho[h][c][r] planes interleaved [c4, L4, c5, L5, ...] (basis change
        # folded into c3 on the host). Chunk 0 is produced at half-batch
        # width; chunks 1-3 at full width (slices feed both matmul passes).
        rho = [[[None] * NPLANES for _ in range(NCHUNK)] for _ in range(2)]

        # The ACT queue must execute tanh/squares strictly in production
        # order — the list scheduler otherwise hoists a later chunk's tanh
        # (gated on its xt DMA) ahead of the current chunk's squares,
        # starving the matmul stream for several us.
        act_chain = [None]

        def chain_act(inst):
            if act_chain[0] is not None:
                add_dep_helper(inst.ins, act_chain[0].ins, sync=False,
                               reason="ACT production order")
            act_chain[0] = inst
            return inst

        def produce_unit(xn_t, c, mi, m, fd_half, prio_ctx, prev_end):
            """Emit one m-unit (two planes) for chunk c. fd_half: 0/1 for a
            half-batch unit on that half, None for full-batch."""
            beta = t_bias - m
            if fd_half is None:
                shape = [128, 2 * BHALF]
            else:
                shape = [128, BHALF]
            with prio_ctx:
                a = aff_pool.tile(shape, F16, tag="ah" if fd_half is not None else "af")
                a_i = nc.vector.tensor_scalar(
                    a[:], xn_t[:], t_scale, beta, ALU.mult, ALU.add
                )
                if prev_end is not None:
                    # Order-only edge: keep the scheduler from interleaving
                    # the next m's ops into the chain that gates the matmul
                    # stream.
                    add_dep_helper(a_i.ins, prev_end.ins, sync=False,
                                   reason="first-chain order")
                # Square on ACT from the DVE-produced affine (bias 0.0 is a
                # framework-registered const AP; custom biases would need
                # extra const memsets + a barrier on the critical prologue).
                sq = sq_pool.tile(shape, F16, tag="sqh" if fd_half is not None else "sqf")
                chain_act(nc.scalar.activation(sq[:], a[:], AF.Square))
                cc = rho_pool.tile(shape, F16, tag=f"rho_c{c}_{2 * mi}_{fd_half}")
                nc.vector.tensor_mul(cc[:], sq[:], a[:])
                lp = rho_pool.tile(shape, F16, tag=f"rho_c{c}_{2 * mi + 1}_{fd_half}")
                lp_i = nc.vector.tensor_scalar_max(lp[:], cc[:], 0.0)
            return lp, cc, lp_i

        # xt DMAs are spread across the sync/vector/scalar HWDGE rings so the
        # transfers overlap instead of queueing behind one ring.
        xt_ring = {0: nc.sync, 1: nc.sync, 2: nc.scalar, 3: nc.gpsimd}

        # --- chunk 0, half 0: half-batch production, priority-pinned ---
        xt00 = xin_pool.tile([128, BHALF], F16, tag="xt00")
        nc.sync.dma_start(xt00[:], xt[0][0])
        xn00 = xn_pool.tile([128, BHALF], F16, tag="xn00")
        chain_act(nc.scalar.activation(xn00[:], xt00[:], AF.Tanh))
        half_planes = [[None] * NPLANES for _ in range(2)]  # chunk0 per half
        prev = None
        for mi, m in enumerate((4, 5, 6, 7)):
            prio = tc.high_priority() if mi == 0 else nullcontext()
            lp, cc, prev = produce_unit(xn00, 0, mi, m, 0, prio,
                                        prev if mi == 1 else None)
            half_planes[0][2 * mi] = cc
            half_planes[0][2 * mi + 1] = lp

        # --- chunks 1-3: full-batch production ---
        for c in range(1, NCHUNK):
            xtc = xin_pool.tile([128, 2 * BHALF], F16, tag=f"xt{c}")
            ring = xt_ring[c]
            ring.dma_start(xtc[:, :BHALF], xt[c][0])
            ring.dma_start(xtc[:, BHALF:], xt[c][1])
            xnc = xn_pool.tile([128, 2 * BHALF], F16, tag=f"xn{c}")
            chain_act(nc.scalar.activation(xnc[:], xtc[:], AF.Tanh))
            for mi, m in enumerate((4, 5, 6, 7)):
                lp, cc, _ = produce_unit(xnc, c, mi, m, None, nullcontext(), None)
                for h in range(2):
                    rho[h][c][2 * mi] = cc[:, h * BHALF:(h + 1) * BHALF]
                    rho[h][c][2 * mi + 1] = lp[:, h * BHALF:(h + 1) * BHALF]

        # --- chunk 0, half 1: half-batch production (needed only by pass 2) ---
        xt01 = xin_pool.tile([128, BHALF], F16, tag="xt01")
        nc.sync.dma_start(xt01[:], xt[0][1])
        xn01 = xn_pool.tile([128, BHALF], F16, tag="xn01")
        chain_act(nc.scalar.activation(xn01[:], xt01[:], AF.Tanh))
        for mi, m in enumerate((4, 5, 6, 7)):
            lp, cc, _ = produce_unit(xn01, 0, mi, m, 1, nullcontext(), None)
            half_planes[1][2 * mi] = cc
            half_planes[1][2 * mi + 1] = lp
        for h in range(2):
            for r in range(NPLANES):
                rho[h][0][r] = half_planes[h][r][:]

        # Dense matmul: 8 PSUM tiles (o_chunk x b_half) accumulated over all
        # 32 (chunk, plane) K-slices. Pass h=0 is K-major (dense PE stream
        # consuming planes in production order); its PSUM eviction + output
        # DMA overlap pass h=1. Pass h=1 is o-major so each o-tile's
        # eviction + DMA trickle out during the remaining matmuls.
        ps = [
            [
                ps_pool.tile([128, 512], F32, tag=f"ps{o}_{h}", name=f"ps{o}_{h}")
                for h in range(2)
            ]
            for o in range(NCHUNK)
        ]
        NK = NCHUNK * NPLANES

        for _ in range(N_WARMUP):
            nc.tensor.matmul(
                ps[0][0][:64, :], wz[:, :64], wz[:], start=True, stop=True
            )

        def emit_copy_out(o, h):
            # Copies alternate ACT/DVE; output DMA triggers alternate the
            # sync/gpsimd HWDGE rings so the tail's trigger issue
            # parallelizes instead of queueing.
            ot = ost_pool.tile([128, 512], F32, tag=f"ot{o}_{h}", name=f"ot{o}_{h}")
            if o % 2 == 0:
                nc.scalar.activation(ot[:], ps[o][h][:], AF.Copy)
            else:
                nc.vector.tensor_copy(ot[:], ps[o][h][:])
            # In pass 2 the LAST tile (o=3) must trigger on the idle sync
            # ring — queueing it behind gpsimd's earlier triggers costs ~0.7us
            # of pure tail.
            even_ring = (o % 2 == 0) if h == 0 else (o % 2 == 1)
            eng = nc.sync if even_ring else nc.gpsimd
            eng.dma_start(out[o][h], ot[:])

        for kk in range(NK):
            c, r = divmod(kk, NPLANES)
            rt = rho[0][c][r]
            for o in range(4):
                lhsT = c3_sb[c][:, r * O_FEAT + o * 128 : r * O_FEAT + (o + 1) * 128]
                nc.tensor.matmul(
                    ps[o][0][:], lhsT, rt, start=(kk == 0), stop=(kk == NK - 1)
                )
        for o in range(4):
            emit_copy_out(o, 0)
        KTAIL = NK - 4
        for kk in range(KTAIL):
            c, r = divmod(kk, NPLANES)
            rt = rho[1][c][r]
            for o in range(4):
                lhsT = c3_sb[c][:, r * O_FEAT + o * 128 : r * O_FEAT + (o + 1) * 128]
                nc.tensor.matmul(
                    ps[o][1][:], lhsT, rt, start=(kk == 0), stop=False
                )
        for o in range(4):
            for kk in range(KTAIL, NK):
                c, r = divmod(kk, NPLANES)
                lhsT = c3_sb[c][:, r * O_FEAT + o * 128 : r * O_FEAT + (o + 1) * 128]
                nc.tensor.matmul(
                    ps[o][1][:],
                    lhsT,
                    rho[1][c][r],
                    start=False,
                    stop=(kk == NK - 1),
                )
            emit_copy_out(o, 1)
    _split_all_waits(nc)
    return nc


# Basis-change: with L_m = (t-m)_+^3, R_m = (m-t)_+^3, c_m = (t-m)^3 = L_m - R_m,
# B3[j](t) = sum_m [WL[j,m]*L_m + WR[j,m]*R_m]/6
#          = sum_m [(WL+WR)[j,m]*L_m - WR[j,m]*c_m]/6
# (binomial divided-difference weights).
_WL = np.array(
    [
        [0, 0, 0, 0],
        [0, 0, 0, 0],
        [0, 0, 0, 0],
        [0, 0, 0, 0],
        [1, -4, 6, -4],
        [0, 1, -4, 6],
        [0, 0, 1, -4],
        [0, 0, 0, 1],
    ],
    dtype=np.float64,
)
_WR = np.array(
    [
        [1, 0, 0, 0],
        [-4, 1, 0, 0],
        [6, -4, 1, 0],
        [-4, 6, -4, 1],
        [0, 0, 0, 0],
        [0, 0, 0, 0],
        [0, 0, 0, 0],
        [0, 0, 0, 0],
    ],
    dtype=np.float64,
)

_nc_cache: dict = {}


def _prepare(x: np.ndarray, coefficients: np.ndarray, grid: np.ndarray):
    x = np.asarray(x, dtype=np.float32)
    coefficients = np.asarray(coefficients, dtype=np.float32)
    grid = np.asarray(grid, dtype=np.float32)

    # Knot-coordinate transform t = (tanh(x) - grid[0]) / h (uniform grid).
    h = float(grid[-1] - grid[0]) / (len(grid) - 1)
    t_scale = 1.0 / h
    t_bias = -float(grid[0]) / h  # t = t_scale * xn + t_bias; here 2.5, 5.5

    key = (round(t_scale, 9), round(t_bias, 9))
    if key not in _nc_cache:
        _nc_cache[key] = _build_nc(t_scale, t_bias)
    nc = _nc_cache[key]

    # Host-side coefficient fold: C3[i, r, o] = sum_j coeff[i,o,j] * W2[j,r],
    # planes interleaved [c4, L4, c5, L5, ...].
    w2 = np.zeros((8, 8), dtype=np.float64)
    for mi in range(4):
        w2[:, 2 * mi] = -_WR[:, mi] / 6.0
        w2[:, 2 * mi + 1] = (_WL[:, mi] + _WR[:, mi]) / 6.0
    c3f = np.einsum("ioj,jr->iro", coefficients.astype(np.float64), w2)
    c3_arr = np.ascontiguousarray(
        c3f.reshape(NCHUNK, 128, NPLANES, O_FEAT)
        .reshape(NCHUNK, 128, NPLANES * O_FEAT)
        .astype(np.float16)
    )

    # xt tile-contiguous: (chunk, half, part, col) per core.
    xt = x.T.astype(np.float16)  # (512, 8192)
    in_maps = []
    for core in range(N_CORES):
        xs = xt[:, core * B_SHARD : (core + 1) * B_SHARD]  # (512, 1024)
        xtc = np.ascontiguousarray(
            xs.reshape(NCHUNK, 128, 2, B_SHARD // 2).transpose(0, 2, 1, 3)
        )  # (4, 2, 128, 512)
        in_maps.append({"xt": xtc, "c3": c3_arr})
    return nc, in_maps


def kernel(x: np.ndarray, coefficients: np.ndarray, grid: np.ndarray) -> np.ndarray:
    nc, in_maps = _prepare(x, coefficients, grid)
    res = run_bass_kernel_spmd(nc, in_maps, list(range(N_CORES)), trace=False)
    outs = []
    for core in range(N_CORES):
        oc = res.results[core]["out"]  # (4, 2, 128, 512) = (o-chunk, half, part, col)
        out_t = oc.transpose(0, 2, 1, 3).reshape(O_FEAT, B_SHARD)  # (512, 1024)
        outs.append(out_t)
    out_full = np.concatenate(outs, axis=1)  # (512, 8192)
    return np.ascontiguousarray(out_full.T).astype(np.float32)
